# revision 2
# baseline (speedup 1.0000x reference)
"""CrossContextAttentiveDecoder Trainium2 kernel.

Sharding: 8 cores = 4 batches x 2 head-groups. Core c handles batch c//2,
heads (c%2)*8..(c%2)*8+8 (E-slice of 512). Each core computes its partial
output projection; host sums the two partials per batch and adds the
(bo + Wo @ bv) constant.

Score transform p = max(exp(s),1) + n*0.01*exp(-500 s^2) uses a first-order
expansion of exp(noise) (error ~2e-5 rel on final output). The gaussian
comes from ActivationFunctionType.Derivative_Erf = (2/sqrt(pi)) exp(-x^2).
Since Exp and Derivative_Erf live in different ACT table sets, the kernel
runs two phases over the scores (re-running the score matmuls) so only two
table loads happen per core.
"""
import math
import numpy as np
import ml_dtypes

B, LQ, LK = 4, 1024, 1024
QD, KVD, E, OD, H = 1024, 512, 1024, 1024, 16
HD = 64
NC_ = 8
HPG = 8       # heads per group/core
ES = 512      # e-slice per core
BF = ml_dtypes.bfloat16

_STATE = {}


def _gen_noise():
    import jax
    import jax.numpy as jnp
    k1, k2 = jax.random.split(jax.random.key(42))
    cpu = jax.devices("cpu")[0]
    with jax.default_device(cpu):
        u = jax.random.normal(k1, (B, H, LQ, LK), jnp.float32)
        v = jax.random.normal(k2, (B, H, LQ, LK), jnp.float32)
        nz = np.asarray(u) - np.asarray(v)
    return nz


def _build():
    import concourse.bass as bass
    import concourse.mybir as mybir
    import concourse.tile as tile
    from concourse import bacc

    F32 = mybir.dt.float32
    BF16 = mybir.dt.bfloat16
    AF = mybir.ActivationFunctionType
    OP = mybir.AluOpType

    nc = bacc.Bacc("TRN2", target_bir_lowering=False, debug=False,
                   num_devices=NC_)

    qt_d = nc.dram_tensor("qt", [QD, LQ], BF16, kind="ExternalInput")
    kt_d = nc.dram_tensor("kt", [KVD, LK], BF16, kind="ExternalInput")
    vt_d = nc.dram_tensor("vt", [KVD, LK], BF16, kind="ExternalInput")
    wq_d = nc.dram_tensor("wq", [QD, ES], BF16, kind="ExternalInput")
    wk_d = nc.dram_tensor("wk", [KVD, ES], BF16, kind="ExternalInput")
    wv_d = nc.dram_tensor("wv", [KVD, ES], BF16, kind="ExternalInput")
    wo_d = nc.dram_tensor("wo", [ES, OD], BF16, kind="ExternalInput")
    bq_d = nc.dram_tensor("bq", [128, 4], F32, kind="ExternalInput")
    bk_d = nc.dram_tensor("bk", [128, 4], F32, kind="ExternalInput")
    nz_d = nc.dram_tensor("nz", [HPG, LK, LQ], BF16, kind="ExternalInput")
    out_d = nc.dram_tensor("out_t", [OD, LQ], F32, kind="ExternalOutput")

    ESC = 1.0 / 8.0                       # exp(s_raw/8)
    GSC = math.sqrt(500.0) / 8.0          # derf(GSC*s_raw) ~ exp(-500 s^2)

    with tile.TileContext(nc) as tc:
        with (
            tc.tile_pool(name="cst", bufs=1) as cst,
            tc.tile_pool(name="ld", bufs=1) as ld,
            tc.tile_pool(name="oasb", bufs=1) as oasb,
            tc.tile_pool(name="nzp", bufs=2) as nzp,
            tc.tile_pool(name="wk_", bufs=2) as wkp,
            tc.tile_pool(name="msc", bufs=2) as msc,
            tc.tile_pool(name="ocp", bufs=3) as ocp,
            tc.tile_pool(name="pss", bufs=2, space="PSUM") as pss,
            tc.tile_pool(name="psa", bufs=2, space="PSUM") as psa,
        ):
            # ---- static loads ----
            qt_sb = ld.tile([128, 8 * LQ], BF16)
            nc.sync.dma_start(qt_sb.rearrange("p (c l) -> p c l", l=LQ), qt_d.rearrange("(c p) l -> p c l", p=128))
            kt_sb = ld.tile([128, 4 * LK], BF16)
            nc.sync.dma_start(kt_sb.rearrange("p (c l) -> p c l", l=LK), kt_d.rearrange("(c p) l -> p c l", p=128))
            vt_sb = ld.tile([128, 4 * LK], BF16)
            nc.sync.dma_start(vt_sb.rearrange("p (c l) -> p c l", l=LK), vt_d.rearrange("(c p) l -> p c l", p=128))
            wq_sb = ld.tile([128, 8 * ES], BF16)
            nc.sync.dma_start(wq_sb.rearrange("p (c e) -> p c e", e=ES), wq_d.rearrange("(c p) e -> p c e", p=128))
            wk_sb = ld.tile([128, 4 * ES], BF16)
            nc.sync.dma_start(wk_sb.rearrange("p (c e) -> p c e", e=ES), wk_d.rearrange("(c p) e -> p c e", p=128))
            wv_sb = ld.tile([128, 4 * ES], BF16)
            nc.sync.dma_start(wv_sb.rearrange("p (c e) -> p c e", e=ES), wv_d.rearrange("(c p) e -> p c e", p=128))
            bq_sb = cst.tile([128, 4], F32)
            nc.sync.dma_start(bq_sb[:], bq_d[:])
            bk_sb = cst.tile([128, 4], F32)
            nc.sync.dma_start(bk_sb[:], bk_d[:])
            wo_sb = cst.tile([128, 4 * OD], BF16)
            nc.sync.dma_start(wo_sb.rearrange("p (c o) -> p c o", o=OD), wo_d.rearrange("(c p) o -> p c o", p=128))

            QT = cst.tile([128, 4 * LQ], BF16)
            KT = cst.tile([128, 4 * LK], BF16)
            VS = cst.tile([128, 8 * 520], BF16)
            On = cst.tile([128, 4 * LQ], BF16)
            nc.vector.memset(VS[:], 1.0)

            # ---- phase 0: projections ----
            for ec in range(4):
                for lc in range(2):
                    qp = pss.tile([128, 1024], F32, tag="sc")
                    for dc in range(8):
                        nc.tensor.matmul(
                            qp[:, :512],
                            wq_sb[:, dc * ES + ec * 128:dc * ES + (ec + 1) * 128],
                            qt_sb[:, dc * LQ + lc * 512:dc * LQ + lc * 512 + 512],
                            start=(dc == 0), stop=(dc == 7))
                    nc.vector.tensor_scalar(
                        QT[:, ec * LQ + lc * 512:ec * LQ + lc * 512 + 512],
                        qp[:, :512], bq_sb[:, ec:ec + 1], None, OP.add)
            for ec in range(4):
                for lc in range(2):
                    kp = pss.tile([128, 1024], F32, tag="sc")
                    for dc in range(4):
                        nc.tensor.matmul(
                            kp[:, :512],
                            wk_sb[:, dc * ES + ec * 128:dc * ES + (ec + 1) * 128],
                            kt_sb[:, dc * LK + lc * 512:dc * LK + lc * 512 + 512],
                            start=(dc == 0), stop=(dc == 3))
                    nc.vector.tensor_scalar(
                        KT[:, ec * LK + lc * 512:ec * LK + lc * 512 + 512],
                        kp[:, :512], bk_sb[:, ec:ec + 1], None, OP.add)
            for kc in range(8):
                vp = pss.tile([128, 1024], F32, tag="sc")
                for dc in range(4):
                    nc.tensor.matmul(
                        vp[:, :512],
                        vt_sb[:, dc * LK + kc * 128:dc * LK + (kc + 1) * 128],
                        wv_sb[:, dc * ES:dc * ES + 512],
                        start=(dc == 0), stop=(dc == 3))
                nc.vector.tensor_copy(
                    VS[:, kc * 520:(kc + 1) * 520]
                    .rearrange("p (h c) -> p h c", c=65)[:, :, 0:64],
                    vp[:, :512].rearrange("p (h c) -> p h c", c=64))

            oa_tiles = []

            def scores(h, kc):
                er, ecl = (h % 2) * 64, (h // 2) * 1024
                sc = pss.tile([128, 1024], F32, tag="sc")
                for qc in range(2):
                    nc.tensor.matmul(
                        sc[:, qc * 512:(qc + 1) * 512],
                        KT[er:er + 64, ecl + kc * 128:ecl + (kc + 1) * 128],
                        QT[er:er + 64, ecl + qc * 512:ecl + qc * 512 + 512],
                        start=True, stop=True)
                return sc

            # ---- phase A: relu-softmax stream (Exp table set) ----
            for h in range(HPG):
                oa = psa.tile([65, 1024], F32, tag="oa")
                for kc in range(8):
                    sc = scores(h, kc)
                    Et = wkp.tile([128, 1024], BF16, tag="E")
                    nc.scalar.activation(Et[:], sc[:], AF.Exp, scale=ESC)
                    Ec = wkp.tile([128, 1024], BF16, tag="Ec")
                    nc.vector.tensor_scalar_max(Ec[:], Et[:], 1.0)
                    for qc in range(2):
                        nc.tensor.matmul(
                            oa[:, qc * 512:(qc + 1) * 512],
                            VS[:, kc * 520 + h * 65:kc * 520 + (h + 1) * 65],
                            Ec[:, qc * 512:(qc + 1) * 512],
                            start=(kc == 0), stop=(kc == 7))
                oa_s = oasb.tile([65, 1024], F32, tag=f"oas{h}")
                nc.vector.tensor_copy(oa_s[:], oa[:])
                oa_tiles.append(oa_s)

            # ---- phase B: gaussian-noise stream (Derivative_Erf set) ----
            for h in range(HPG):
                nz = nzp.tile([128, 8 * LQ], BF16, tag="nz")
                nc.sync.dma_start(
                    nz.rearrange("p (c q) -> p c q", q=LQ),
                    nz_d[h].rearrange("(c p) q -> p c q", p=128))
                ob = psa.tile([65, 1024], F32, tag="oa")
                for kc in range(8):
                    sc = scores(h, kc)
                    gg = wkp.tile([128, 1024], BF16, tag="E")
                    nc.scalar.activation(gg[:], sc[:], AF.Derivative_Erf,
                                         scale=GSC)
                    hh = wkp.tile([128, 1024], BF16, tag="Ec")
                    nc.vector.tensor_tensor(
                        hh[:], gg[:], nz[:, kc * LQ:(kc + 1) * LQ], OP.mult)
                    for qc in range(2):
                        nc.tensor.matmul(
                            ob[:, qc * 512:(qc + 1) * 512],
                            VS[:, kc * 520 + h * 65:kc * 520 + (h + 1) * 65],
                            hh[:, qc * 512:(qc + 1) * 512],
                            start=(kc == 0), stop=(kc == 7))
                # merge + normalize
                oa_s = oa_tiles[h]
                dm = msc.tile([1, 1024], F32, tag="dm")
                nc.vector.tensor_tensor(dm[:], ob[64:65, :], oa_s[64:65, :],
                                        OP.add)
                rr = msc.tile([1, 1024], F32, tag="rr")
                nc.vector.reciprocal_approx_fast(rr[:], dm[:])
                Rb = msc.tile([64, 1024], F32, tag="Rb")
                nc.gpsimd.partition_broadcast(Rb[:], rr[:])
                om = msc.tile([64, 1024], F32, tag="om")
                nc.vector.tensor_tensor(om[:], ob[0:64, :], oa_s[0:64, :],
                                        OP.add)
                er, ecl = (h % 2) * 64, (h // 2) * 1024
                nc.vector.tensor_tensor(
                    On[er:er + 64, ecl:ecl + 1024], om[:], Rb[:], OP.mult)

            # ---- phase C: output projection ----
            for oc in range(8):
                for lc in range(2):
                    op_ps = pss.tile([128, 1024], F32, tag="sc")
                    for ec in range(4):
                        nc.tensor.matmul(
                            op_ps[:, :512],
                            wo_sb[:, ec * OD + oc * 128:ec * OD + (oc + 1) * 128],
                            On[:, ec * LQ + lc * 512:ec * LQ + lc * 512 + 512],
                            start=(ec == 0), stop=(ec == 3))
                    oc_sb = ocp.tile([128, 512], F32, tag="ocp")
                    nc.scalar.copy(oc_sb[:], op_ps[:, :512])
                    nc.sync.dma_start(
                        out_d[oc * 128:(oc + 1) * 128, lc * 512:(lc + 1) * 512],
                        oc_sb[:])

    nc.compile()
    return nc


def _prep(query, key_x, value, Wq, bq, Wk, bk, Wv, bv, Wo, bo):
    nscale = 0.01 * math.sqrt(math.pi) / 2.0
    noise = _gen_noise() * nscale
    in_maps = []
    for c in range(NC_):
        b, g = c // 2, c % 2
        es = slice(g * ES, (g + 1) * ES)
        m = dict(
            qt=np.ascontiguousarray(query[b].T).astype(BF),
            kt=np.ascontiguousarray(key_x[b].T).astype(BF),
            vt=np.ascontiguousarray(value[b].T).astype(BF),
            wq=np.ascontiguousarray(Wq[es].T).astype(BF),
            wk=np.ascontiguousarray(Wk[es].T).astype(BF),
            wv=np.ascontiguousarray(Wv[es].T).astype(BF),
            wo=np.ascontiguousarray(Wo[:, es].T).astype(BF),
            bq=np.ascontiguousarray(bq[es].reshape(4, 128).T).astype(np.float32),
            bk=np.ascontiguousarray(bk[es].reshape(4, 128).T).astype(np.float32),
            nz=np.ascontiguousarray(
                noise[b, g * HPG:(g + 1) * HPG].swapaxes(1, 2)).astype(BF),
        )
        in_maps.append(m)
    return in_maps


def kernel(query, key_x, value, Wq, bq, Wk, bk, Wv, bv, Wo, bo):
    from concourse import bass_utils
    if "nc" not in _STATE:
        _STATE["nc"] = _build()
    nc = _STATE["nc"]
    in_maps = _prep(query, key_x, value, Wq, bq, Wk, bk, Wv, bv, Wo, bo)
    res = bass_utils.run_bass_kernel_spmd(nc, in_maps,
                                          core_ids=list(range(NC_)))
    cvec = (bo + Wo @ bv).astype(np.float32)
    out = np.empty((B, LQ, OD), np.float32)
    for b in range(B):
        pt = res.results[2 * b]["out_t"] + res.results[2 * b + 1]["out_t"]
        out[b] = pt.T + cvec
    return out
